# revision 45
# baseline (speedup 1.0000x reference)
"""LocallyConnected2d Trainium2 kernel (bf16 pipeline).

Problem: out[b,o,oh,ow] = sum_{c,ki,kj} x[b,c,oh+ki,ow+kj] * W[o,oh,ow,c,ki,kj] + bias[o,oh,ow]
Shapes: x[32,32,64,64], W[64,62,62,32,3,3], bias[64,62,62] -> out[32,64,62,62], fp32 I/O.

The untied weight tensor (283 MB fp32) is read exactly once -> the kernel is
HBM-bandwidth bound. All operands ship as bf16 (accuracy gate 2e-2 vs ~2.7e-3
measured bf16 error), halving the dominant weight stream; PSUM accumulates in
fp32; the output returns as bf16 and is upcast on host.

Strategy (8 NeuronCores, sharded over output rows, 8 rows/core padded to 64):
- Per output location: 3 accumulating PE matmuls, K=97 each (chunk q = kernel
  row ki; features j=(kj,c) plus a ones-row at j=96 that carries bias on q=2).
- lhsT (stationary) = x patch columns [97,32b]: x ships unshifted [c,h,w64,b]
  (1.3 MB) and is replicated on-chip into 3 column-shifted replicas on
  partitions kj*32+c via SBUF->SBUF DMA, so every lhsT is a direct AP slice.
  Partition 96 is memset to 1.0 (carries the bias row).
- rhs (moving) = per-location weights [97,64o], streamed from HBM in
  half-row strips (11.9KB padded lines). DMA shape lessons (measured):
  SWDGE chops lines 16-way into ~744B packets (~97 GB/s); HWDGE fans a DMA
  across the 16 SDMA engines only when the line count is divisible by 16,
  else the whole transfer lands on ONE engine (~27 GB/s). So: 96-line
  feature DMAs + separate bias line, alternating between the two HWDGE
  rings (sync/scalar), all weight DMAs emitted before any out-store.
- One PSUM bank [128,512] per strip accumulates 8 location-groups (4
  locations x 32b on partitions, 64o per group in free); one DVE copy casts
  the bank into a persistent bf16 out tile; out-stores go in multi-strip
  chunks with fat per-partition lines.
"""

import numpy as np
import ml_dtypes

import concourse.bass as bass  # noqa: F401
import concourse.mybir as mybir
import concourse.tile as tile
from concourse import bacc
from concourse.bass_utils import run_bass_kernel_spmd

B, C_IN, H, W = 32, 32, 64, 64
C_OUT, OH, OW, KK = 64, 62, 62, 3
N_CORES = 8
ROWS = 8          # padded output rows per core (8*8=64 >= 62)
HALF = 31         # locations per strip (half an output row)
XH = ROWS + 2     # input rows needed per core
KP = 97           # contraction per chunk: 96 features + ones/bias row
NG = 8            # ceil(31/4) location groups per strip
SLINE = 3 * HALF * C_OUT  # 5952 weight elems per strip per j-line
WLINE = SLINE + 32        # padded line (non-contiguous source)
F32 = mybir.dt.float32
BF16 = mybir.dt.bfloat16
NP_BF16 = ml_dtypes.bfloat16

_NC_CACHE = {}


def _build_nc():
    nc = bacc.Bacc(
        "TRN2",
        target_bir_lowering=False,
        debug=False,
        enable_asserts=False,
        num_devices=N_CORES,
    )
    # x ships host-transposed AND pre-shifted into 3 kj-replicas
    # [kj, c, h, w(62), b]: on-chip replication was tried (SWDGE and HWDGE
    # sb2sb) and lost — sb2sb consumes the same per-engine descriptor
    # cadence that bounds the HBM stream, and delays the first strips.
    x_d = nc.dram_tensor("x", [KK, C_IN, XH, OW, B], BF16, kind="ExternalInput").ap()
    w_d = nc.dram_tensor(
        "w", [ROWS, 2, KP, WLINE], BF16, kind="ExternalInput"
    ).ap()
    # out layout: [p=(l4,b), strip, grp, o] - partition-major; host
    # unscrambles + upcasts
    o_d = nc.dram_tensor(
        "out", [128, ROWS * 2 * NG * C_OUT], BF16, kind="ExternalOutput"
    ).ap()

    with tile.TileContext(nc) as tc:
        with (
            tc.tile_pool(name="xpool", bufs=1) as xpool,
            # bufs=8 measured numerically WRONG on HW (rel err 0.69, likely
            # SBUF allocation overflow) with no speed gain — keep 6
            tc.tile_pool(name="wpool", bufs=6) as wpool,
            tc.tile_pool(name="opool", bufs=1) as opool,
            # bufs=4 measured best (3: 128.7us, 4: 128.0us, 6: 129.1us)
            tc.tile_pool(name="pspool", bufs=4, space="PSUM") as pspool,
        ):
            HZ = OW * B  # 1984 elems per h-row
            # x in THREE row-range tiles: matmul waits proved to be
            # tile-granular, so with one x3 tile the first strips waited on
            # x rows they never read. Each matmul touches exactly one h-row
            # (row+q), so row-range tiles split cleanly.
            XROWS = ((0, 3), (3, 6), (6, XH))
            x3s = [
                xpool.tile([KP, (r1 - r0) * HZ], BF16, name=f"x3_{r0}")
                for r0, r1 in XROWS
            ]
            for t in x3s:
                # partition 96 = 1.0 (carries the bias row); memset instead
                # of a DRAM ones-load
                nc.vector.memset(t[96:97, :], 1.0)
            xsrc = x_d.rearrange("k c h w b -> (k c) (h w b)")

            def xrow(r):
                # (tile, base) for input row r
                for ti, (r0, r1) in enumerate(XROWS):
                    if r < r1:
                        return x3s[ti], (r - r0) * HZ
                raise AssertionError

            def load_x_rows(ti, eng):
                r0, r1 = XROWS[ti]
                eng.dma_start(
                    out=x3s[ti][0:96, :],
                    in_=xsrc[0:96, r0 * HZ : r1 * HZ],
                )

            # x rides the scalar ring (ahead of the odd strips' weights) so
            # w0 starts streaming on sync immediately
            load_x_rows(0, nc.scalar)

            QZ = HALF * C_OUT  # 1984, one chunk per kernel row q
            # all weight DMAs emitted up-front (wpool bufs provide the
            # streaming backpressure) so out-stores never head-of-line
            # block the weight stream on either ring. Strips 13/15 join the
            # sync ring to offset scalar's x + out-store load (mid-stream
            # rebalance {5,9} measured no better).
            wts = []
            for s in range(2 * ROWS):
                weng = nc.sync if (s % 2 == 0 or s >= 13) else nc.scalar
                wt_full = wpool.tile([KP, WLINE], BF16, tag="wt")
                wsrc = w_d[s // 2, s % 2]
                # partition 96 (ones/bias) only joins the q=2 matmul (the
                # q<2 matmuls use K=96 APs), so only the q=2 third of the
                # bias line ships — the q<2 thirds are zeros by construction
                if s == 15:
                    # last strip: stream in 4 location-quarters (2 groups
                    # each), each quarter fully before the next, so earlier
                    # quarters' matmuls drain while later bytes are still in
                    # flight (wt waits are region-granular)
                    PQ = 8 * C_OUT  # 512 cols per quarter
                    for p in range(4):
                        c0, c1 = p * PQ, min((p + 1) * PQ, QZ)
                        weng.dma_start(
                            out=wt_full[96:97, 2 * QZ + c0 : 2 * QZ + c1],
                            in_=wsrc[96:97, 2 * QZ + c0 : 2 * QZ + c1],
                        )
                        for q in range(3):
                            weng.dma_start(
                                out=wt_full[0:96, q * QZ + c0 : q * QZ + c1],
                                in_=wsrc[0:96, q * QZ + c0 : q * QZ + c1],
                            )
                else:
                    weng.dma_start(
                        out=wt_full[96:97, 2 * QZ : 3 * QZ],
                        in_=wsrc[96:97, 2 * QZ : 3 * QZ],
                    )
                    if s < 2:
                        # split the first strip on each ring by q-chunk so
                        # its first matmuls unblock after 1/3 of the strip
                        for f0, f1 in ((0, QZ), (QZ, 2 * QZ), (2 * QZ, 3 * QZ)):
                            weng.dma_start(
                                out=wt_full[0:96, f0:f1], in_=wsrc[0:96, f0:f1]
                            )
                    else:
                        weng.dma_start(
                            out=wt_full[0:96, 0:SLINE], in_=wsrc[0:96, 0:SLINE]
                        )
                wts.append(wt_full)
                if s == 1:
                    load_x_rows(1, nc.scalar)
                elif s == 3:
                    load_x_rows(2, nc.scalar)

            SZ = NG * C_OUT  # 512 out elems per strip per partition
            ot = opool.tile([128, 2 * ROWS * SZ], BF16)  # all strips
            OUT_CHUNKS = {3: (0, 4), 7: (4, 8), 11: (8, 12), 14: (12, 15)}
            for s in range(2 * ROWS):
                row, half = s // 2, s % 2
                wt = wts[s]
                # one PSUM bank per strip: partitions (l4,b), free (grp, o)
                ps = pspool.tile([128, SZ], F32, tag="ps")
                # (g, li, q) order only: q-outer wave orders produced WRONG
                # results on HW (start=True clears a PSUM zero-region wider
                # than the matmul's output slice — the CoreSim checker's
                # "pending group in zero region" error is real).
                # Last strip: copy + store per location-part right behind
                # the quarter-streamed weights to minimize the final drain.
                for g, li, q in (
                    (g, li, q) for g in range(NG) for li in range(4) for q in range(3)
                ):
                    gn = min(4, HALF - g * 4)  # 4,4,...,3
                    # pad slot in the last group duplicates the prior
                    # location (keeps PSUM fully written; host drops it)
                    eff = min(li, gn - 1)
                    ow = half * HALF + g * 4 + eff
                    loff = (g * 4 + eff) * C_OUT
                    xt, xbase = xrow(row + q)
                    kp = KP if q == 2 else 96  # ones/bias row only on q=2
                    nc.tensor.matmul(
                        ps[32 * li : 32 * li + 32, g * C_OUT : (g + 1) * C_OUT],
                        xt[
                            0:kp, xbase + ow * B : xbase + ow * B + B
                        ],  # [kp, 32] lhsT
                        wt[0:kp, q * QZ + loff : q * QZ + loff + C_OUT],
                        start=(q == 0),
                        stop=(q == 2),
                        tile_position=(0, 32 * li),
                    )
                if s == 15:
                    # per-part copies chase the quarter-streamed matmuls;
                    # two half-stores so only 24 MMs + a 128-col copy + a
                    # 64KB store remain after the last weight byte
                    for p in range(4):
                        nc.vector.tensor_copy(
                            out=ot[:, s * SZ + 128 * p : s * SZ + 128 * (p + 1)],
                            in_=ps[:, 128 * p : 128 * (p + 1)],
                        )
                        if p % 2 == 1:
                            h0 = s * SZ + 256 * (p // 2)
                            nc.scalar.dma_start(
                                out=o_d[:, h0 : h0 + 256], in_=ot[:, h0 : h0 + 256]
                            )
                else:
                    nc.vector.tensor_copy(out=ot[:, s * SZ : (s + 1) * SZ], in_=ps)
                    if s in OUT_CHUNKS:
                        c0, c1 = OUT_CHUNKS[s]
                        nc.scalar.dma_start(
                            out=o_d[:, c0 * SZ : c1 * SZ], in_=ot[:, c0 * SZ : c1 * SZ]
                        )

    nc.compile()
    return nc


def get_nc():
    if "nc" not in _NC_CACHE:
        _NC_CACHE["nc"] = _build_nc()
    return _NC_CACHE["nc"]


def prep_inputs(x, weight, bias):
    """Host-side shard + layout prep. Returns per-core in_maps."""
    x = np.asarray(x, dtype=np.float32)
    weight = np.asarray(weight, dtype=np.float32)
    bias = np.asarray(bias, dtype=np.float32)

    # w_prep[oh, j=kj*32+c, q=ki, ow, o]; j=96 row: 0 for q<2, bias for q=2
    wp = np.zeros((N_CORES * ROWS, KP, 3, OW, C_OUT), NP_BF16)
    wp[:OH, :96] = (
        weight.transpose(1, 5, 3, 4, 2, 0).reshape(OH, 96, 3, OW, C_OUT)
    ).astype(NP_BF16)
    wp[:OH, 96, 2] = bias.transpose(1, 2, 0).astype(NP_BF16)
    # half-row strips with padded lines: [row, half, j, (q l o)+32]
    wp = wp.reshape(N_CORES * ROWS, KP, 3, 2, HALF, C_OUT).transpose(0, 3, 1, 2, 4, 5)
    wpad = np.zeros((N_CORES * ROWS, 2, KP, WLINE), NP_BF16)
    wpad[:, :, :, :SLINE] = wp.reshape(N_CORES * ROWS, 2, KP, SLINE)
    wp = wpad

    # x pre-shifted into 3 kj-replicas [kj, c, h, w(62), b]
    xp = np.zeros((B, C_IN, N_CORES * ROWS + 2, W), NP_BF16)
    xp[:, :, :H] = x.astype(NP_BF16)
    xt = xp.transpose(1, 2, 3, 0)  # [c, h, w, b]

    in_maps = []
    for c in range(N_CORES):
        r0 = c * ROWS
        xc = xt[:, r0 : r0 + XH]  # [c, 10, 64, b]
        xsh = np.stack([xc[:, :, kj : kj + OW, :] for kj in range(KK)])
        in_maps.append(
            {
                "x": np.ascontiguousarray(xsh),
                "w": np.ascontiguousarray(wp[r0 : r0 + ROWS]),
            }
        )
    return in_maps


def gather_output(results):
    """results: list of per-core out dicts -> full [B, C_OUT, OH, OW] fp32."""
    out = np.empty((B, C_OUT, OH, OW), np.float32)
    for c in range(N_CORES):
        # out[p=(l4,b), (strip, grp, o)]
        oc = np.asarray(results[c]["out"]).astype(np.float32)
        v = oc.reshape(4, B, ROWS, 2, NG, C_OUT)
        # ow = half*31 + grp*4 + l  (grp*4+l < 31)
        arr = v.transpose(1, 5, 2, 3, 4, 0).reshape(B, C_OUT, ROWS, 2, 32)
        arr = arr[:, :, :, :, :HALF].reshape(B, C_OUT, ROWS, OW)
        r0 = c * ROWS
        rows = min(ROWS, OH - r0)
        out[:, :, r0 : r0 + rows, :] = arr[:, :, :rows, :]
    return out


def run(inputs, **kw):
    nc = get_nc()
    in_maps = prep_inputs(inputs["x"], inputs["weight"], inputs["bias"])
    res = run_bass_kernel_spmd(nc, in_maps, core_ids=list(range(N_CORES)), **kw)
    return gather_output(res.results), res


def kernel(x, weight, bias):
    out, _ = run({"x": x, "weight": weight, "bias": bias})
    return out


# revision 50
# speedup vs baseline: 1.0516x; 1.0516x over previous
"""LocallyConnected2d Trainium2 kernel (bf16 pipeline).

Problem: out[b,o,oh,ow] = sum_{c,ki,kj} x[b,c,oh+ki,ow+kj] * W[o,oh,ow,c,ki,kj] + bias[o,oh,ow]
Shapes: x[32,32,64,64], W[64,62,62,32,3,3], bias[64,62,62] -> out[32,64,62,62], fp32 I/O.

The untied weight tensor (283 MB fp32) is read exactly once -> the kernel is
HBM-bandwidth bound. All operands ship as bf16 (accuracy gate 2e-2 vs ~2.7e-3
measured bf16 error), halving the dominant weight stream; PSUM accumulates in
fp32; the output returns as bf16 and is upcast on host.

Strategy (8 NeuronCores, sharded over output rows, 8 rows/core padded to 64):
- Per output location: 3 accumulating PE matmuls, K=97 each (chunk q = kernel
  row ki; features j=(kj,c) plus a ones-row at j=96 that carries bias on q=2).
- lhsT (stationary) = x patch columns [97,32b]: x ships unshifted [c,h,w64,b]
  (1.3 MB) and is replicated on-chip into 3 column-shifted replicas on
  partitions kj*32+c via SBUF->SBUF DMA, so every lhsT is a direct AP slice.
  Partition 96 is memset to 1.0 (carries the bias row).
- rhs (moving) = per-location weights [97,64o], streamed from HBM in
  half-row strips (11.9KB padded lines). DMA shape lessons (measured):
  SWDGE chops lines 16-way into ~744B packets (~97 GB/s); HWDGE fans a DMA
  across the 16 SDMA engines only when the line count is divisible by 16,
  else the whole transfer lands on ONE engine (~27 GB/s). So: 96-line
  feature DMAs + separate bias line, alternating between the two HWDGE
  rings (sync/scalar), all weight DMAs emitted before any out-store.
- One PSUM bank [128,512] per strip accumulates 8 location-groups (4
  locations x 32b on partitions, 64o per group in free); one DVE copy casts
  the bank into a persistent bf16 out tile; out-stores go in multi-strip
  chunks with fat per-partition lines.
"""

import numpy as np
import ml_dtypes

import concourse.bass as bass  # noqa: F401
import concourse.mybir as mybir
import concourse.tile as tile
from concourse import bacc
from concourse.bass_utils import run_bass_kernel_spmd

B, C_IN, H, W = 32, 32, 64, 64
C_OUT, OH, OW, KK = 64, 62, 62, 3
N_CORES = 8
ROWS = 8          # padded output rows per core (8*8=64 >= 62)
HALF = 31         # locations per strip (half an output row)
XH = ROWS + 2     # input rows needed per core
KP = 97           # contraction per chunk: 96 features + ones/bias row
NG = 8            # ceil(31/4) location groups per strip
SLINE = 3 * HALF * C_OUT  # 5952 weight elems per strip per j-line
WLINE = SLINE + 32        # padded line (non-contiguous source)
F32 = mybir.dt.float32
BF16 = mybir.dt.bfloat16
NP_BF16 = ml_dtypes.bfloat16

_NC_CACHE = {}


def _build_nc():
    nc = bacc.Bacc(
        "TRN2",
        target_bir_lowering=False,
        debug=False,
        enable_asserts=False,
        num_devices=N_CORES,
    )
    # x ships host-transposed AND pre-shifted into 3 kj-replicas
    # [kj, c, h, w(62), b]: on-chip replication was tried (SWDGE and HWDGE
    # sb2sb) and lost — sb2sb consumes the same per-engine descriptor
    # cadence that bounds the HBM stream, and delays the first strips.
    x_d = nc.dram_tensor("x", [KK, C_IN, XH, OW, B], BF16, kind="ExternalInput").ap()
    w_d = nc.dram_tensor(
        "w", [ROWS, 2, KP, WLINE], BF16, kind="ExternalInput"
    ).ap()
    # out layout: [p=(l4,b), strip, grp, o] - partition-major; host
    # unscrambles + upcasts
    o_d = nc.dram_tensor(
        "out", [128, ROWS * 2 * NG * C_OUT], BF16, kind="ExternalOutput"
    ).ap()

    with tile.TileContext(nc) as tc:
        with (
            tc.tile_pool(name="xpool", bufs=1) as xpool,
            # bufs=8 measured numerically WRONG on HW (rel err 0.69, likely
            # SBUF allocation overflow) with no speed gain — keep 6
            tc.tile_pool(name="wpool", bufs=6) as wpool,
            tc.tile_pool(name="opool", bufs=1) as opool,
            # bufs=4 measured best (3: 128.7us, 4: 128.0us, 6: 129.1us)
            tc.tile_pool(name="pspool", bufs=4, space="PSUM") as pspool,
        ):
            HZ = OW * B  # 1984 elems per h-row
            # x in THREE row-range tiles: matmul waits proved to be
            # tile-granular, so with one x3 tile the first strips waited on
            # x rows they never read. Each matmul touches exactly one h-row
            # (row+q), so row-range tiles split cleanly.
            XROWS = ((0, 3), (3, 6), (6, XH))
            x3s = [
                xpool.tile([KP, (r1 - r0) * HZ], BF16, name=f"x3_{r0}")
                for r0, r1 in XROWS
            ]
            for t in x3s:
                # partition 96 = 1.0 (carries the bias row); memset instead
                # of a DRAM ones-load
                nc.vector.memset(t[96:97, :], 1.0)
            xsrc = x_d.rearrange("k c h w b -> (k c) (h w b)")

            def xrow(r):
                # (tile, base) for input row r
                for ti, (r0, r1) in enumerate(XROWS):
                    if r < r1:
                        return x3s[ti], (r - r0) * HZ
                raise AssertionError

            def load_x_rows(ti, eng):
                r0, r1 = XROWS[ti]
                eng.dma_start(
                    out=x3s[ti][0:96, :],
                    in_=xsrc[0:96, r0 * HZ : r1 * HZ],
                )

            # x rides the scalar ring (ahead of the odd strips' weights) so
            # w0 starts streaming on sync immediately
            load_x_rows(0, nc.scalar)

            QZ = HALF * C_OUT  # 1984, one chunk per kernel row q
            # all weight DMAs emitted up-front (wpool bufs provide the
            # streaming backpressure) so out-stores never head-of-line
            # block the weight stream on either ring. Strips 13/15 join the
            # sync ring to offset scalar's x + out-store load (mid-stream
            # rebalance {5,9} measured no better).
            wts = []
            for s in range(2 * ROWS):
                weng = nc.sync if (s % 2 == 0 or s >= 13) else nc.scalar
                wt_full = wpool.tile([KP, WLINE], BF16, tag="wt")
                wsrc = w_d[s // 2, s % 2]
                # partition 96 (ones/bias) only joins the q=2 matmul (the
                # q<2 matmuls use K=96 APs), so only the q=2 third of the
                # bias line ships — the q<2 thirds are zeros by construction
                if s == 15:
                    # last strip: stream location-half A (groups 0-3, first
                    # 16*64 cols of each q chunk) fully before half B, so
                    # half A's matmuls drain while B is still in flight
                    # (wt waits are region-granular). Quarter-splitting plus
                    # per-part copies/stores measured WORSE (133.4us).
                    HA = 16 * C_OUT
                    weng.dma_start(
                        out=wt_full[96:97, 2 * QZ : 2 * QZ + HA],
                        in_=wsrc[96:97, 2 * QZ : 2 * QZ + HA],
                    )
                    for q in range(3):
                        weng.dma_start(
                            out=wt_full[0:96, q * QZ : q * QZ + HA],
                            in_=wsrc[0:96, q * QZ : q * QZ + HA],
                        )
                    weng.dma_start(
                        out=wt_full[96:97, 2 * QZ + HA : 3 * QZ],
                        in_=wsrc[96:97, 2 * QZ + HA : 3 * QZ],
                    )
                    for q in range(3):
                        weng.dma_start(
                            out=wt_full[0:96, q * QZ + HA : (q + 1) * QZ],
                            in_=wsrc[0:96, q * QZ + HA : (q + 1) * QZ],
                        )
                else:
                    weng.dma_start(
                        out=wt_full[96:97, 2 * QZ : 3 * QZ],
                        in_=wsrc[96:97, 2 * QZ : 3 * QZ],
                    )
                    if s < 2:
                        # split the first strip on each ring by q-chunk so
                        # its first matmuls unblock after 1/3 of the strip
                        for f0, f1 in ((0, QZ), (QZ, 2 * QZ), (2 * QZ, 3 * QZ)):
                            weng.dma_start(
                                out=wt_full[0:96, f0:f1], in_=wsrc[0:96, f0:f1]
                            )
                    else:
                        weng.dma_start(
                            out=wt_full[0:96, 0:SLINE], in_=wsrc[0:96, 0:SLINE]
                        )
                wts.append(wt_full)
                if s == 1:
                    load_x_rows(1, nc.scalar)
                elif s == 3:
                    load_x_rows(2, nc.scalar)

            SZ = NG * C_OUT  # 512 out elems per strip per partition
            ot = opool.tile([128, 2 * ROWS * SZ], BF16)  # all strips
            OUT_CHUNKS = {3: (0, 4), 7: (4, 8), 11: (8, 12), 14: (12, 15)}
            for s in range(2 * ROWS):
                row, half = s // 2, s % 2
                wt = wts[s]
                # one PSUM bank per strip: partitions (l4,b), free (grp, o)
                ps = pspool.tile([128, SZ], F32, tag="ps")
                # (g, li, q) order only: q-outer wave orders produced WRONG
                # results on HW (start=True clears a PSUM zero-region wider
                # than the matmul's output slice — the CoreSim checker's
                # "pending group in zero region" error is real).
                # Last strip: copy + store per location-part right behind
                # the quarter-streamed weights to minimize the final drain.
                for g, li, q in (
                    (g, li, q) for g in range(NG) for li in range(4) for q in range(3)
                ):
                    gn = min(4, HALF - g * 4)  # 4,4,...,3
                    # pad slot in the last group duplicates the prior
                    # location (keeps PSUM fully written; host drops it)
                    eff = min(li, gn - 1)
                    ow = half * HALF + g * 4 + eff
                    loff = (g * 4 + eff) * C_OUT
                    xt, xbase = xrow(row + q)
                    kp = KP if q == 2 else 96  # ones/bias row only on q=2
                    nc.tensor.matmul(
                        ps[32 * li : 32 * li + 32, g * C_OUT : (g + 1) * C_OUT],
                        xt[
                            0:kp, xbase + ow * B : xbase + ow * B + B
                        ],  # [kp, 32] lhsT
                        wt[0:kp, q * QZ + loff : q * QZ + loff + C_OUT],
                        start=(q == 0),
                        stop=(q == 2),
                        tile_position=(0, 32 * li),
                    )
                if s == 15:
                    # copy+store in A/B halves matching the A/B weight
                    # stream: half A's result ships while half B's matmuls
                    # finish, leaving only a 256-col copy + 64KB store
                    # after the last matmul
                    for h in range(2):
                        nc.vector.tensor_copy(
                            out=ot[:, s * SZ + 256 * h : s * SZ + 256 * (h + 1)],
                            in_=ps[:, 256 * h : 256 * (h + 1)],
                        )
                        nc.scalar.dma_start(
                            out=o_d[:, s * SZ + 256 * h : s * SZ + 256 * (h + 1)],
                            in_=ot[:, s * SZ + 256 * h : s * SZ + 256 * (h + 1)],
                        )
                else:
                    nc.vector.tensor_copy(out=ot[:, s * SZ : (s + 1) * SZ], in_=ps)
                    if s in OUT_CHUNKS:
                        c0, c1 = OUT_CHUNKS[s]
                        nc.scalar.dma_start(
                            out=o_d[:, c0 * SZ : c1 * SZ], in_=ot[:, c0 * SZ : c1 * SZ]
                        )

    nc.compile()
    return nc


def get_nc():
    if "nc" not in _NC_CACHE:
        _NC_CACHE["nc"] = _build_nc()
    return _NC_CACHE["nc"]


def prep_inputs(x, weight, bias):
    """Host-side shard + layout prep. Returns per-core in_maps."""
    x = np.asarray(x, dtype=np.float32)
    weight = np.asarray(weight, dtype=np.float32)
    bias = np.asarray(bias, dtype=np.float32)

    # w_prep[oh, j=kj*32+c, q=ki, ow, o]; j=96 row: 0 for q<2, bias for q=2
    wp = np.zeros((N_CORES * ROWS, KP, 3, OW, C_OUT), NP_BF16)
    wp[:OH, :96] = (
        weight.transpose(1, 5, 3, 4, 2, 0).reshape(OH, 96, 3, OW, C_OUT)
    ).astype(NP_BF16)
    wp[:OH, 96, 2] = bias.transpose(1, 2, 0).astype(NP_BF16)
    # half-row strips with padded lines: [row, half, j, (q l o)+32]
    wp = wp.reshape(N_CORES * ROWS, KP, 3, 2, HALF, C_OUT).transpose(0, 3, 1, 2, 4, 5)
    wpad = np.zeros((N_CORES * ROWS, 2, KP, WLINE), NP_BF16)
    wpad[:, :, :, :SLINE] = wp.reshape(N_CORES * ROWS, 2, KP, SLINE)
    wp = wpad

    # x pre-shifted into 3 kj-replicas [kj, c, h, w(62), b]
    xp = np.zeros((B, C_IN, N_CORES * ROWS + 2, W), NP_BF16)
    xp[:, :, :H] = x.astype(NP_BF16)
    xt = xp.transpose(1, 2, 3, 0)  # [c, h, w, b]

    in_maps = []
    for c in range(N_CORES):
        r0 = c * ROWS
        xc = xt[:, r0 : r0 + XH]  # [c, 10, 64, b]
        xsh = np.stack([xc[:, :, kj : kj + OW, :] for kj in range(KK)])
        in_maps.append(
            {
                "x": np.ascontiguousarray(xsh),
                "w": np.ascontiguousarray(wp[r0 : r0 + ROWS]),
            }
        )
    return in_maps


def gather_output(results):
    """results: list of per-core out dicts -> full [B, C_OUT, OH, OW] fp32."""
    out = np.empty((B, C_OUT, OH, OW), np.float32)
    for c in range(N_CORES):
        # out[p=(l4,b), (strip, grp, o)]
        oc = np.asarray(results[c]["out"]).astype(np.float32)
        v = oc.reshape(4, B, ROWS, 2, NG, C_OUT)
        # ow = half*31 + grp*4 + l  (grp*4+l < 31)
        arr = v.transpose(1, 5, 2, 3, 4, 0).reshape(B, C_OUT, ROWS, 2, 32)
        arr = arr[:, :, :, :, :HALF].reshape(B, C_OUT, ROWS, OW)
        r0 = c * ROWS
        rows = min(ROWS, OH - r0)
        out[:, :, r0 : r0 + rows, :] = arr[:, :, :rows, :]
    return out


def run(inputs, **kw):
    nc = get_nc()
    in_maps = prep_inputs(inputs["x"], inputs["weight"], inputs["bias"])
    res = run_bass_kernel_spmd(nc, in_maps, core_ids=list(range(N_CORES)), **kw)
    return gather_output(res.results), res


def kernel(x, weight, bias):
    out, _ = run({"x": x, "weight": weight, "bias": bias})
    return out


# revision 52
# speedup vs baseline: 1.0556x; 1.0038x over previous
"""LocallyConnected2d Trainium2 kernel (bf16 pipeline).

Problem: out[b,o,oh,ow] = sum_{c,ki,kj} x[b,c,oh+ki,ow+kj] * W[o,oh,ow,c,ki,kj] + bias[o,oh,ow]
Shapes: x[32,32,64,64], W[64,62,62,32,3,3], bias[64,62,62] -> out[32,64,62,62], fp32 I/O.

The untied weight tensor (283 MB fp32) is read exactly once -> the kernel is
HBM-bandwidth bound. All operands ship as bf16 (accuracy gate 2e-2 vs ~2.7e-3
measured bf16 error), halving the dominant weight stream; PSUM accumulates in
fp32; the output returns as bf16 and is upcast on host.

Strategy (8 NeuronCores, sharded over output rows, 8 rows/core padded to 64):
- Per output location: 3 accumulating PE matmuls, K=97 each (chunk q = kernel
  row ki; features j=(kj,c) plus a ones-row at j=96 that carries bias on q=2).
- lhsT (stationary) = x patch columns [97,32b]: x ships unshifted [c,h,w64,b]
  (1.3 MB) and is replicated on-chip into 3 column-shifted replicas on
  partitions kj*32+c via SBUF->SBUF DMA, so every lhsT is a direct AP slice.
  Partition 96 is memset to 1.0 (carries the bias row).
- rhs (moving) = per-location weights [97,64o], streamed from HBM in
  half-row strips (11.9KB padded lines). DMA shape lessons (measured):
  SWDGE chops lines 16-way into ~744B packets (~97 GB/s); HWDGE fans a DMA
  across the 16 SDMA engines only when the line count is divisible by 16,
  else the whole transfer lands on ONE engine (~27 GB/s). So: 96-line
  feature DMAs + separate bias line, alternating between the two HWDGE
  rings (sync/scalar), all weight DMAs emitted before any out-store.
- One PSUM bank [128,512] per strip accumulates 8 location-groups (4
  locations x 32b on partitions, 64o per group in free); one DVE copy casts
  the bank into a persistent bf16 out tile; out-stores go in multi-strip
  chunks with fat per-partition lines.
"""

import numpy as np
import ml_dtypes

import concourse.bass as bass  # noqa: F401
import concourse.mybir as mybir
import concourse.tile as tile
from concourse import bacc
from concourse.bass_utils import run_bass_kernel_spmd

B, C_IN, H, W = 32, 32, 64, 64
C_OUT, OH, OW, KK = 64, 62, 62, 3
N_CORES = 8
ROWS = 8          # padded output rows per core (8*8=64 >= 62)
HALF = 31         # locations per strip (half an output row)
XH = ROWS + 2     # input rows needed per core
KP = 97           # contraction per chunk: 96 features + ones/bias row
NG = 8            # ceil(31/4) location groups per strip
SLINE = 3 * HALF * C_OUT  # 5952 weight elems per strip per j-line
WLINE = SLINE + 32        # padded line (non-contiguous source)
F32 = mybir.dt.float32
BF16 = mybir.dt.bfloat16
NP_BF16 = ml_dtypes.bfloat16

_NC_CACHE = {}


def _build_nc():
    nc = bacc.Bacc(
        "TRN2",
        target_bir_lowering=False,
        debug=False,
        enable_asserts=False,
        num_devices=N_CORES,
    )
    # x ships host-transposed AND pre-shifted into 3 kj-replicas
    # [kj, c, h, w(62), b]: on-chip replication was tried (SWDGE and HWDGE
    # sb2sb) and lost — sb2sb consumes the same per-engine descriptor
    # cadence that bounds the HBM stream, and delays the first strips.
    x_d = nc.dram_tensor("x", [KK, C_IN, XH, OW, B], BF16, kind="ExternalInput").ap()
    w_d = nc.dram_tensor(
        "w", [ROWS, 2, KP, WLINE], BF16, kind="ExternalInput"
    ).ap()
    # out layout: [p=(l4,b), strip, grp, o] - partition-major; host
    # unscrambles + upcasts
    o_d = nc.dram_tensor(
        "out", [128, ROWS * 2 * NG * C_OUT], BF16, kind="ExternalOutput"
    ).ap()

    with tile.TileContext(nc) as tc:
        with (
            tc.tile_pool(name="xpool", bufs=1) as xpool,
            # bufs=8 measured numerically WRONG on HW (rel err 0.69, likely
            # SBUF allocation overflow) with no speed gain — keep 6
            tc.tile_pool(name="wpool", bufs=6) as wpool,
            tc.tile_pool(name="opool", bufs=1) as opool,
            # bufs=4 measured best (3: 128.7us, 4: 128.0us, 6: 129.1us)
            tc.tile_pool(name="pspool", bufs=4, space="PSUM") as pspool,
        ):
            HZ = OW * B  # 1984 elems per h-row
            # x in THREE row-range tiles: matmul waits proved to be
            # tile-granular, so with one x3 tile the first strips waited on
            # x rows they never read. Each matmul touches exactly one h-row
            # (row+q), so row-range tiles split cleanly.
            XROWS = ((0, 3), (3, 6), (6, XH))
            x3s = [
                xpool.tile([KP, (r1 - r0) * HZ], BF16, name=f"x3_{r0}")
                for r0, r1 in XROWS
            ]
            for t in x3s:
                # partition 96 = 1.0 (carries the bias row); memset instead
                # of a DRAM ones-load
                nc.vector.memset(t[96:97, :], 1.0)
            xsrc = x_d.rearrange("k c h w b -> (k c) (h w b)")

            def xrow(r):
                # (tile, base) for input row r
                for ti, (r0, r1) in enumerate(XROWS):
                    if r < r1:
                        return x3s[ti], (r - r0) * HZ
                raise AssertionError

            def load_x_rows(ti, eng):
                r0, r1 = XROWS[ti]
                eng.dma_start(
                    out=x3s[ti][0:96, :],
                    in_=xsrc[0:96, r0 * HZ : r1 * HZ],
                )

            # x rides the scalar ring (ahead of the odd strips' weights) so
            # w0 starts streaming on sync immediately
            load_x_rows(0, nc.scalar)

            QZ = HALF * C_OUT  # 1984, one chunk per kernel row q
            # all weight DMAs emitted up-front (wpool bufs provide the
            # streaming backpressure) so out-stores never head-of-line
            # block the weight stream on either ring. Strips 13/15 join the
            # sync ring to offset scalar's x + out-store load (mid-stream
            # rebalance {5,9} measured no better).
            wts = []
            for s in range(2 * ROWS):
                weng = nc.sync if (s % 2 == 0 or s >= 13) else nc.scalar
                wt_full = wpool.tile([KP, WLINE], BF16, tag="wt")
                wsrc = w_d[s // 2, s % 2]
                # partition 96 (ones/bias) only joins the q=2 matmul (the
                # q<2 matmuls use K=96 APs), so only the q=2 third of the
                # bias line ships — the q<2 thirds are zeros by construction
                if s == 15:
                    # last strip: stream location-half A (groups 0-3, first
                    # 16*64 cols of each q chunk) fully before half B, so
                    # half A's matmuls drain while B is still in flight
                    # (wt waits are region-granular). Quarter-splitting plus
                    # per-part copies/stores measured WORSE (133.4us).
                    HA = 16 * C_OUT
                    weng.dma_start(
                        out=wt_full[96:97, 2 * QZ : 2 * QZ + HA],
                        in_=wsrc[96:97, 2 * QZ : 2 * QZ + HA],
                    )
                    for q in range(3):
                        weng.dma_start(
                            out=wt_full[0:96, q * QZ : q * QZ + HA],
                            in_=wsrc[0:96, q * QZ : q * QZ + HA],
                        )
                    weng.dma_start(
                        out=wt_full[96:97, 2 * QZ + HA : 3 * QZ],
                        in_=wsrc[96:97, 2 * QZ + HA : 3 * QZ],
                    )
                    for q in range(3):
                        weng.dma_start(
                            out=wt_full[0:96, q * QZ + HA : (q + 1) * QZ],
                            in_=wsrc[0:96, q * QZ + HA : (q + 1) * QZ],
                        )
                else:
                    weng.dma_start(
                        out=wt_full[96:97, 2 * QZ : 3 * QZ],
                        in_=wsrc[96:97, 2 * QZ : 3 * QZ],
                    )
                    if s < 2:
                        # split the first strip on each ring by q-chunk so
                        # its first matmuls unblock after 1/3 of the strip
                        for f0, f1 in ((0, QZ), (QZ, 2 * QZ), (2 * QZ, 3 * QZ)):
                            weng.dma_start(
                                out=wt_full[0:96, f0:f1], in_=wsrc[0:96, f0:f1]
                            )
                    else:
                        weng.dma_start(
                            out=wt_full[0:96, 0:SLINE], in_=wsrc[0:96, 0:SLINE]
                        )
                wts.append(wt_full)
                if s == 1:
                    load_x_rows(1, nc.scalar)
                elif s == 3:
                    load_x_rows(2, nc.scalar)

            SZ = NG * C_OUT  # 512 out elems per strip per partition
            ot = opool.tile([128, 2 * ROWS * SZ], BF16)  # all strips
            OUT_CHUNKS = {3: (0, 4), 7: (4, 8), 11: (8, 12), 14: (12, 15), 15: (15, 16)}
            for s in range(2 * ROWS):
                row, half = s // 2, s % 2
                wt = wts[s]
                # one PSUM bank per strip: partitions (l4,b), free (grp, o)
                ps = pspool.tile([128, SZ], F32, tag="ps")
                # (g, li, q) order only: q-outer wave orders produced WRONG
                # results on HW (start=True clears a PSUM zero-region wider
                # than the matmul's output slice — the CoreSim checker's
                # "pending group in zero region" error is real).
                # Last strip: copy + store per location-part right behind
                # the quarter-streamed weights to minimize the final drain.
                for g, li, q in (
                    (g, li, q) for g in range(NG) for li in range(4) for q in range(3)
                ):
                    gn = min(4, HALF - g * 4)  # 4,4,...,3
                    # pad slot in the last group duplicates the prior
                    # location (keeps PSUM fully written; host drops it)
                    eff = min(li, gn - 1)
                    ow = half * HALF + g * 4 + eff
                    loff = (g * 4 + eff) * C_OUT
                    xt, xbase = xrow(row + q)
                    kp = KP if q == 2 else 96  # ones/bias row only on q=2
                    nc.tensor.matmul(
                        ps[32 * li : 32 * li + 32, g * C_OUT : (g + 1) * C_OUT],
                        xt[
                            0:kp, xbase + ow * B : xbase + ow * B + B
                        ],  # [kp, 32] lhsT
                        wt[0:kp, q * QZ + loff : q * QZ + loff + C_OUT],
                        start=(q == 0),
                        stop=(q == 2),
                        tile_position=(0, 32 * li),
                    )
                # single copy + store per strip (A/B-split tail copies
                # measured worse: PSUM reads are tile-granular so the
                # halves serialize — 126,807 vs 125,937)
                nc.vector.tensor_copy(out=ot[:, s * SZ : (s + 1) * SZ], in_=ps)
                if s in OUT_CHUNKS:
                    c0, c1 = OUT_CHUNKS[s]
                    nc.scalar.dma_start(
                        out=o_d[:, c0 * SZ : c1 * SZ], in_=ot[:, c0 * SZ : c1 * SZ]
                    )

    nc.compile()
    return nc


def get_nc():
    if "nc" not in _NC_CACHE:
        _NC_CACHE["nc"] = _build_nc()
    return _NC_CACHE["nc"]


def prep_inputs(x, weight, bias):
    """Host-side shard + layout prep. Returns per-core in_maps."""
    x = np.asarray(x, dtype=np.float32)
    weight = np.asarray(weight, dtype=np.float32)
    bias = np.asarray(bias, dtype=np.float32)

    # w_prep[oh, j=kj*32+c, q=ki, ow, o]; j=96 row: 0 for q<2, bias for q=2
    wp = np.zeros((N_CORES * ROWS, KP, 3, OW, C_OUT), NP_BF16)
    wp[:OH, :96] = (
        weight.transpose(1, 5, 3, 4, 2, 0).reshape(OH, 96, 3, OW, C_OUT)
    ).astype(NP_BF16)
    wp[:OH, 96, 2] = bias.transpose(1, 2, 0).astype(NP_BF16)
    # half-row strips with padded lines: [row, half, j, (q l o)+32]
    wp = wp.reshape(N_CORES * ROWS, KP, 3, 2, HALF, C_OUT).transpose(0, 3, 1, 2, 4, 5)
    wpad = np.zeros((N_CORES * ROWS, 2, KP, WLINE), NP_BF16)
    wpad[:, :, :, :SLINE] = wp.reshape(N_CORES * ROWS, 2, KP, SLINE)
    wp = wpad

    # x pre-shifted into 3 kj-replicas [kj, c, h, w(62), b]
    xp = np.zeros((B, C_IN, N_CORES * ROWS + 2, W), NP_BF16)
    xp[:, :, :H] = x.astype(NP_BF16)
    xt = xp.transpose(1, 2, 3, 0)  # [c, h, w, b]

    in_maps = []
    for c in range(N_CORES):
        r0 = c * ROWS
        xc = xt[:, r0 : r0 + XH]  # [c, 10, 64, b]
        xsh = np.stack([xc[:, :, kj : kj + OW, :] for kj in range(KK)])
        in_maps.append(
            {
                "x": np.ascontiguousarray(xsh),
                "w": np.ascontiguousarray(wp[r0 : r0 + ROWS]),
            }
        )
    return in_maps


def gather_output(results):
    """results: list of per-core out dicts -> full [B, C_OUT, OH, OW] fp32."""
    out = np.empty((B, C_OUT, OH, OW), np.float32)
    for c in range(N_CORES):
        # out[p=(l4,b), (strip, grp, o)]
        oc = np.asarray(results[c]["out"]).astype(np.float32)
        v = oc.reshape(4, B, ROWS, 2, NG, C_OUT)
        # ow = half*31 + grp*4 + l  (grp*4+l < 31)
        arr = v.transpose(1, 5, 2, 3, 4, 0).reshape(B, C_OUT, ROWS, 2, 32)
        arr = arr[:, :, :, :, :HALF].reshape(B, C_OUT, ROWS, OW)
        r0 = c * ROWS
        rows = min(ROWS, OH - r0)
        out[:, :, r0 : r0 + rows, :] = arr[:, :, :rows, :]
    return out


def run(inputs, **kw):
    nc = get_nc()
    in_maps = prep_inputs(inputs["x"], inputs["weight"], inputs["bias"])
    res = run_bass_kernel_spmd(nc, in_maps, core_ids=list(range(N_CORES)), **kw)
    return gather_output(res.results), res


def kernel(x, weight, bias):
    out, _ = run({"x": x, "weight": weight, "bias": bias})
    return out
